# revision 9
# baseline (speedup 1.0000x reference)
"""Causal self-attention (4, 2048, 1024), 16 heads, on 8 trn2 NeuronCores.

Sharding: batch (4) x head-group (2 groups of 8 heads) -> 8 cores.
Each core computes, for its batch b and its 8 heads:
  qkv projection -> causal attention -> partial output projection
  partial_out = Y_heads @ w_proj[rows of those heads]
Host sums the two head-group partials per batch. No collectives.

Per-core kernel (all matmuls fp32r):
  phase A: QT/KT in (d, T) layout + V in (T, d) layout from xT and sliced
           weights (contraction over c=1024 on partitions).
  phase B: per head-pair, per 512-query group: scores^T blocks [k,q] via
           row-tiled concurrent matmuls (two heads in PE rows 0:64/64:128),
           exp on ACT (scale=1/8 folded), causal diag masking via mask mult,
           P@V with a ones-extended V (M=128: 64 YT rows + 64 identical
           softmax-denominator rows) accumulated in PSUM.
  phase C: output projection from YT layout, contraction over the pair dim.
"""

import os
import numpy as np

import concourse.bass as bass
import concourse.mybir as mybir
import concourse.tile as tile
from concourse import bacc

F32 = mybir.dt.float32
F32R = mybir.dt.float32r

T = 2048  # sequence length
C = 1024  # embed dim
NH = 8    # heads per core
D = 64    # head dim
NP = 4    # head pairs per core
NKT = 16  # k-tiles of 128
NQG = 4   # query groups of 512


def r(ap):
    return ap.bitcast(F32R)


def build_nc():
    nc = bacc.Bacc(trn_type="TRN2", target_bir_lowering=False, debug=False,
                   num_devices=8)
    xT = nc.dram_tensor("xT", [C, T], F32, kind="ExternalInput").ap()
    # wqkv cols: [q: 8 heads x 64 | k: 8 heads x 64 | v: 4 even heads x 64,
    #             4 odd heads x 64]  (v part pre-permuted on host)
    wqkv = nc.dram_tensor("wqkv", [C, 3 * 512], F32, kind="ExternalInput").ap()
    wproj = nc.dram_tensor("wproj", [512, C], F32, kind="ExternalInput").ap()
    # mask2[k, i, q] = 1.0 if q >= k else 0 (same for i=0,1)
    mask2 = nc.dram_tensor("mask2", [128, 2, 128], F32, kind="ExternalInput").ap()
    ones64 = nc.dram_tensor("ones64", [128, 64], F32, kind="ExternalInput").ap()
    out = nc.dram_tensor("out", [T, C], F32, kind="ExternalOutput").ap()

    with tile.TileContext(nc) as tc:
        build_body(tc, xT, wqkv, wproj, mask2, ones64, out)
    nc.compile()
    return nc


def build_body(tc, xT, wqkv, wproj, mask2, ones64, out):
    nc = tc.nc
    import contextlib
    ctx = contextlib.ExitStack()
    with ctx:
        persist = ctx.enter_context(tc.tile_pool(name="persist", bufs=1))
        qt_t = persist.tile([128, NP, T], F32R)       # QT pairs (d=128, t)
        kt_t = persist.tile([128, NP, T], F32R)       # KT pairs
        # per (ktile, head) 128-col block: even heads [V|ones], odd [ones|V]
        vv_t = persist.tile([128, NKT, 8, 128], F32R)
        mask_t = persist.tile([128, 2, 128], F32R)

        nc.sync.dma_start(out=mask_t[:], in_=mask2[:].bitcast(F32R))
        # fill the ones halves of vv via broadcast DMAs from the ones input
        vv_ap = vv_t[:, :, :, :]
        pstep = vv_ap.ap[0][0]
        ones_bc = bass.AP(tensor=ones64.tensor, offset=0,
                          ap=[[64, 128], [0, 4], [1, 64]]).bitcast(F32R)
        for l in range(NKT):
            # even heads: ones at cols 64:128; odd heads: ones at cols 0:64
            evens = bass.AP(tensor=vv_ap.tensor,
                            offset=vv_ap.offset + 1024 * l + 64,
                            ap=[[pstep, 128], [256, 4], [1, 64]])
            nc.sync.dma_start(out=evens, in_=ones_bc)
            odds = bass.AP(tensor=vv_ap.tensor,
                           offset=vv_ap.offset + 1024 * l + 128,
                           ap=[[pstep, 128], [256, 4], [1, 64]])
            nc.sync.dma_start(out=odds, in_=ones_bc)

        # ---------------- phase A: QKV projections ----------------
        with tc.tile_pool(name="ph_a", bufs=1) as pa, \
             tc.tile_pool(name="xt_p", bufs=2) as xtp, \
             tc.tile_pool(name="w_p", bufs=2) as wap, \
             tc.tile_pool(name="ps_a", bufs=4, space="PSUM") as psa:
            wv_t = pa.tile([128, 8, 512], F32R)
            nc.sync.dma_start(
                out=wv_t[:],
                in_=wqkv[:, 1024:1536].rearrange("(c p) n -> p c n", p=128).bitcast(F32R))

            for tq in range(4):  # t-quarters of 512
                xt = xtp.tile([128, 8, 512], F32R)
                for c in range(8):
                    nc.sync.dma_start(
                        out=xt[:, c, :],
                        in_=xT[128 * c:128 * (c + 1),
                               512 * tq:512 * (tq + 1)].bitcast(F32R))
                for m in range(8):  # 0-3 QT pairs, 4-7 KT pairs
                    w_m = wap.tile([128, 8, 128], F32R, tag="wm")
                    nc.sync.dma_start(
                        out=w_m[:],
                        in_=wqkv[:, 128 * m:128 * (m + 1)].rearrange(
                            "(c p) n -> p c n", p=128).bitcast(F32R))
                    ps = psa.tile([128, 512], F32, tag="qkps")
                    for c in range(8):
                        nc.tensor.matmul(ps[:], r(w_m[:, c, :]),
                                         r(xt[:, c, :]),
                                         start=(c == 0), stop=(c == 7))
                    dst = qt_t if m < 4 else kt_t
                    nc.vector.tensor_copy(
                        dst[:, m % 4, 512 * tq:512 * (tq + 1)], ps[:])
                for tt in range(4):  # V for the 4 t-tiles of this quarter
                    ps = psa.tile([128, 512], F32, tag="vps")
                    for c in range(8):
                        nc.tensor.matmul(ps[:], r(xt[:, c, 128 * tt:128 * (tt + 1)]),
                                         r(wv_t[:, c, :]),
                                         start=(c == 0), stop=(c == 7))
                    l = 4 * tq + tt
                    psr = ps[:].rearrange("p (hp par d) -> p hp par d",
                                          par=2, d=64)
                    vv4 = vv_t[:, l, :, :].rearrange(
                        "p (hp par) d -> p hp par d", par=2)
                    nc.vector.tensor_copy(vv4[:, :, 0, 0:64], psr[:, :, 0, :])
                    nc.vector.tensor_copy(vv4[:, :, 1, 64:128], psr[:, :, 1, :])

        # ---------------- phases B+C ----------------
        late = ctx.enter_context(tc.tile_pool(name="late", bufs=1))
        yt_t = late.tile([128, NP, T], F32R)          # normalized Y^T pairs
        # ---------------- phase B: causal attention ----------------
        with tc.tile_pool(name="e_p", bufs=3) as ep, \
             tc.tile_pool(name="rep_p", bufs=3) as rpp, \
             tc.tile_pool(name="st_p", bufs=2, space="PSUM") as stp, \
             tc.tile_pool(name="yt_ps", bufs=3, space="PSUM") as ytp:
            for g in range(NP):
                for j in range(NQG):
                    q0 = 512 * j
                    yA = ytp.tile([128, 512], F32, tag="ytps")
                    yB = ytp.tile([128, 512], F32, tag="ytps")
                    hA, hB = 2 * g, 2 * g + 1
                    blocks = list(range(4 * j, 4 * j + 4)) + list(range(0, 4 * j))
                    nb = len(blocks)
                    for idx, l in enumerate(blocks):
                        off = 128 * (l - 4 * j) if l >= 4 * j else 0
                        st = stp.tile([128, 2, 512], F32, tag="st")
                        nc.tensor.matmul(st[:, 0, off:512],
                                         r(kt_t[0:64, g, 128 * l:128 * (l + 1)]),
                                         r(qt_t[0:64, g, q0 + off:q0 + 512]),
                                         start=True, stop=True)
                        nc.tensor.matmul(st[:, 1, off:512],
                                         r(kt_t[64:128, g, 128 * l:128 * (l + 1)]),
                                         r(qt_t[64:128, g, q0 + off:q0 + 512]),
                                         start=True, stop=True)
                        e = ep.tile([128, 2, 512], F32R, tag="e")
                        nc.scalar.activation(e[:, :, off:512], st[:, :, off:512],
                                             mybir.ActivationFunctionType.Exp,
                                             scale=0.125)
                        if l >= 4 * j:
                            nc.vector.tensor_mul(e[:, :, off:off + 128],
                                                 e[:, :, off:off + 128],
                                                 mask_t[:])
                        fl = dict(start=(idx == 0), stop=(idx == nb - 1))
                        # head A: yA rows 0:64 = YT_A, rows 64:128 = sums_A
                        nc.tensor.matmul(yA[:, off:512],
                                         r(vv_t[:, l, hA, :]),
                                         r(e[:, 0, off:512]), **fl)
                        # head B: yB rows 0:64 = sums_B, rows 64:128 = YT_B
                        nc.tensor.matmul(yB[:, off:512],
                                         r(vv_t[:, l, hB, :]),
                                         r(e[:, 1, off:512]), **fl)
                    # epilogue: normalize by softmax denominators
                    repA = rpp.tile([128, 512], F32, tag="rep")
                    nc.vector.reciprocal(repA[64:128, :], yA[64:128, :])
                    nc.sync.dma_start(out=repA[0:64, :], in_=repA[64:128, :])
                    nc.vector.tensor_mul(yt_t[0:64, g, q0:q0 + 512],
                                         yA[0:64, :], repA[0:64, :])
                    repB = rpp.tile([128, 512], F32, tag="rep")
                    nc.vector.reciprocal(repB[0:64, :], yB[0:64, :])
                    nc.sync.dma_start(out=repB[64:128, :], in_=repB[0:64, :])
                    nc.vector.tensor_mul(yt_t[64:128, g, q0:q0 + 512],
                                         yB[64:128, :], repB[64:128, :])

        # ---------------- phase C: output projection ----------------
        with tc.tile_pool(name="ph_c", bufs=1) as pc, \
             tc.tile_pool(name="osb_p", bufs=2) as osp, \
             tc.tile_pool(name="ps_c", bufs=4, space="PSUM") as psc:
            wp_t = pc.tile([128, NP, C], F32R)
            nc.sync.dma_start(
                out=wp_t[:],
                in_=wproj.rearrange("(g p) n -> p g n", p=128).bitcast(F32R))
            for tt in range(16):
                for ec in range(2):
                    ps = psc.tile([128, 512], F32, tag="pps")
                    for g in range(NP):
                        nc.tensor.matmul(ps[:],
                                         r(yt_t[:, g, 128 * tt:128 * (tt + 1)]),
                                         r(wp_t[:, g, 512 * ec:512 * (ec + 1)]),
                                         start=(g == 0), stop=(g == 3))
                    ob = osp.tile([128, 512], F32, tag="ob")
                    nc.scalar.copy(ob[:], ps[:])
                    nc.sync.dma_start(
                        out=out[128 * tt:128 * (tt + 1), 512 * ec:512 * (ec + 1)],
                        in_=ob[:])


def make_core_inputs(x, w_attn, w_proj):
    """Host-side sharding: returns list of 8 input dicts."""
    x = np.asarray(x, dtype=np.float32)
    w_attn = np.asarray(w_attn, dtype=np.float32)
    w_proj = np.asarray(w_proj, dtype=np.float32)
    k = np.arange(128)
    m = (k[None, :] >= k[:, None]).astype(np.float32)  # [k, q] keep if q >= k
    mask2 = np.ascontiguousarray(np.stack([m, m], axis=1))  # [128, 2, 128]
    in_maps = []
    for core in range(8):
        b, hg = divmod(core, 2)
        cs = 512 * hg
        wq = w_attn[:, cs:cs + 512]
        wk = w_attn[:, 1024 + cs:1024 + cs + 512]
        wv = w_attn[:, 2048 + cs:2048 + cs + 512]
        wqkv = np.ascontiguousarray(np.concatenate([wq, wk, wv], axis=1))
        in_maps.append({
            "xT": np.ascontiguousarray(x[b].T),
            "wqkv": wqkv,
            "wproj": np.ascontiguousarray(w_proj[cs:cs + 512, :]),
            "mask2": mask2,
            "ones64": np.ones((128, 64), dtype=np.float32),
        })
    return in_maps


_NC_CACHE = {}


def get_nc():
    if "nc" not in _NC_CACHE:
        _NC_CACHE["nc"] = build_nc()
    return _NC_CACHE["nc"]


def kernel(x, w_attn, w_proj):
    from concourse.bass_utils import run_bass_kernel_spmd
    nc = get_nc()
    in_maps = make_core_inputs(x, w_attn, w_proj)
    res = run_bass_kernel_spmd(nc, in_maps, list(range(8)), trace=False)
    parts = [res.results[i]["out"] for i in range(8)]
    y = np.stack([parts[2 * b] + parts[2 * b + 1] for b in range(4)], axis=0)
    return y.astype(np.float32)
